# revision 54
# baseline (speedup 1.0000x reference)
"""Bahdanau-attention kernel for Trainium2, data-parallel over batch on 8 cores.

Shapes (full): h_encoder [32, 2048, 1024], h_decoder_prev [32, 1024],
W1 [2048, 1024], b1 [1024], W2 [1024, 1], b2 [1].
Returns (context_vector [32, 1024] f32, attention_weights [32, 2048, 1] f32).

Per-core (4 batch elements each), all matmuls in bf16 with f32 PSUM accum:
  E1^T[u,s] = tanh(W1_enc^T . h^T + dec_proj[b] + b1)   (TensorE + ScalarE)
  e2[s]     = relu(W2^T . E1^T + b2)                    (TensorE + ScalarE)
  w[s]      = softmax_s(e2)                             (ScalarE exp + VectorE)
  ctx[d]    = sum_s w[s] h[b,s,d]                       (VectorE mult-reduce on h^T)

Host prep (cheap, no FLOP-relevant work): shard by batch, transpose
h_encoder to [d, s] per batch, split W1, cast to bf16.
"""

import numpy as np

B, S, D, U = 32, 2048, 1024, 1024
W1E_SCALE = 32.0
NF8 = 6  # u-chunks computed in fp8 DoubleRow; rest bf16
N_CORES = 8
BL = B // N_CORES  # 4 batch elements per core
S512 = 512
NJ = S // S512  # 4 s-chunks
NC_ = 8  # number of 128-wide chunks in D / U


def _patch_tile_drain(tile, mybir):
    """This walrus build rejects >1 sync wait on one CTRL instruction; split
    the Tile tail-drain's waits one-per-nop."""
    from concourse.vector_clock import ScopedClock

    if getattr(tile.TileContext, "_drain_patched", False):
        return

    def _patched(self, tick_clock, wait_clock):
        nc = self.nc
        drain_inst = nc.sync.drain()
        wait_clock.add_sem_waits(
            drain_inst.ins, ScopedClock({None: tick_clock.global_clock})
        )
        si = drain_inst.ins.sync_info
        if si is not None and si.on_wait and len(si.on_wait) > 1:
            extras = list(si.on_wait[1:])
            si.on_wait = [si.on_wait[0]]
            for w in extras:
                nop = nc.sync.nop(nofuse=True, hint="drain_wait_split")
                nsi = nop.ins.sync_info
                if nsi is None:
                    nop.ins.sync_info = mybir.SyncInfo(on_wait=[w], on_update=[])
                else:
                    nsi.on_wait = [w]
        nc.all_engine_barrier()
        popped = nc._tile_sem_poison_stack.pop()
        assert popped is self._sem_poison
        # clear_and_free_semaphores emits one RANGE_CLEAR ISA instruction for
        # the whole range, which this walrus rejects ("ISA wrong length") for
        # wide ranges — chunk it.
        sems = list(self.sems.allocated().values())
        sem_nums = sorted(
            s.num if hasattr(s, "num") else int(s) for s in sems
        )
        CH = 8
        for i in range(0, len(sem_nums), CH):
            chunk = sem_nums[i : i + CH]
            from concourse.bass import compact_to_ranges

            for r in compact_to_ranges(chunk):
                nc.gpsimd.dma_reset(r)
                nc.gpsimd.sem_clear(r)

    tile.TileContext._drain_and_barrier = _patched
    tile.TileContext._drain_patched = True


_COMPUTE_ENGINES = ("PE", "Activation", "DVE", "Pool", "SP")


def _dedup_ldweights(nc, mybir):
    """Drop an LDWEIGHTS that reloads the exact weights of the immediately
    preceding LDWEIGHTS on the PE stream.  The following matmul then uses the
    already-loaded stationary (HW preserves program-order weight semantics).
    Only drops sync-free instances."""
    dropped = 0
    for fn in nc.m.functions:
        for bb in fn.blocks:
            last_sig = None
            keep = []
            for inst in bb.instructions:
                nm = type(inst).__name__
                if nm == "InstLdweights":
                    sig = (
                        str(inst.ins[0]),
                        str(getattr(inst, "perf_mode", None)),
                        str(getattr(inst, "is_transpose", None)),
                    )
                    si = inst.sync_info
                    clean = si is None or (not si.on_wait and not si.on_update)
                    if sig == last_sig and clean:
                        dropped += 1
                        continue
                    last_sig = sig
                keep.append(inst)
            if len(keep) != len(bb.instructions):
                bb.instructions[:] = keep
    return dropped


def _split_multi_waits(nc, mybir, max_waits=1):
    """This walrus build rejects instructions carrying more than one sync
    wait; hoist extras onto nops inserted just before, on the same engine."""
    idx = 0
    eng_ok = {getattr(mybir.EngineType, n) for n in _COMPUTE_ENGINES}
    for fn in nc.m.functions:
        for bb in fn.blocks:
            new_insts = []
            changed = False
            for inst in bb.instructions:
                si = inst.sync_info
                if (
                    si is not None
                    and si.on_wait
                    and len(si.on_wait) > max_waits
                    and inst.engine in eng_ok
                ):
                    waits = list(si.on_wait)
                    extras, keep = waits[:-max_waits], waits[-max_waits:]
                    for w in extras:
                        idx += 1
                        new_insts.append(
                            mybir.InstNoOp(
                                name=nc.get_next_instruction_name(),
                                engine=inst.engine,
                                ins=[],
                                outs=[],
                                sync_info=mybir.SyncInfo(on_wait=[w], on_update=[]),
                                bass_nofuse=True,
                                text_hint="waitsplit",
                            )
                        )
                    si.on_wait = keep
                    changed = True
                new_insts.append(inst)
            if changed:
                try:
                    bb.instructions[:] = new_insts
                except TypeError:
                    bb.instructions = new_insts


def _build_nc():
    import concourse.bass as bass
    import concourse.mybir as mybir
    import concourse.tile as tile

    _patch_tile_drain(tile, mybir)

    bf = mybir.dt.bfloat16
    f32 = mybir.dt.float32
    fp8 = mybir.dt.float8e4
    AF = mybir.ActivationFunctionType
    ALU = mybir.AluOpType
    AX = mybir.AxisListType

    nc = bass.Bass()

    ht8_p = nc.declare_dram_parameter("ht8", [BL, 128, NC_ * S], fp8, isOutput=False)
    htb_p = nc.declare_dram_parameter("htb", [BL, 128, NC_ * S], bf, isOutput=False)
    hdect_p = nc.declare_dram_parameter("hdect", [128, NC_ * BL], bf, isOutput=False)
    w1e8_p = nc.declare_dram_parameter("w1e8", [128, NF8 * U], fp8, isOutput=False)
    w1eb_p = nc.declare_dram_parameter("w1eb", [128, (NC_ - NF8) * U], bf, isOutput=False)
    w1d_p = nc.declare_dram_parameter("w1d", [128, NC_ * U], bf, isOutput=False)
    w2_p = nc.declare_dram_parameter("w2", [128, NC_], bf, isOutput=False)
    b1_p = nc.declare_dram_parameter("b1", [128, NC_], f32, isOutput=False)
    b2_p = nc.declare_dram_parameter("b2", [1, 1], f32, isOutput=False)
    ones_p = nc.declare_dram_parameter("ones", [1, 128], bf, isOutput=False)
    onesf_p = nc.declare_dram_parameter("onesf", [1, 128], f32, isOutput=False)
    ctx_p = nc.declare_dram_parameter("ctx", [128, BL * NC_], f32, isOutput=True)
    attn_p = nc.declare_dram_parameter("attn", [BL, S], f32, isOutput=True)

    with tile.TileContext(nc) as tc:
        with (
            tc.tile_pool(name="const", bufs=1) as const,
            tc.tile_pool(name="hpool", bufs=8) as hpool,
            tc.tile_pool(name="e1pool", bufs=6) as e1pool,
            tc.tile_pool(name="wbpool", bufs=3) as wbpool,
            tc.tile_pool(name="scrpool", bufs=4) as scrpool,
            tc.tile_pool(name="rows", bufs=2) as rows,
            tc.tile_pool(name="psA", bufs=4, space="PSUM") as psA,
            tc.tile_pool(name="psB", bufs=2, space="PSUM") as psB,
            tc.tile_pool(name="psC", bufs=2, space="PSUM") as psC,
        ):
            # ---- constants into SBUF ----
            # Weights are laid out c-major on host ([p, c*U + k*128 + m]) so
            # one 256KB chunk carries a full contraction for one u-chunk.
            # DMA issue order = first-compute order: the first psum tile only
            # needs w1e chunk c0 + ht(b0) slice j0 (~1.25MB), so those go
            # first; then the dec_proj deps, then the rest.
            JW = NC_ * S512  # 4096: width of one j-major slice
            # Deps are effectively tile-granular, so give every
            # independently-consumed chunk its own tile.  The first psum tile
            # needs w1e chunk c0 + ht(b0) slice j0 (~1.25MB): issue those
            # first, split across several dma_starts so all HWDGE queues
            # work on the critical bytes before anything else is queued.
            # dec_proj heads the PE stream and the first tanh needs its
            # result (bias), so w1d + hdect + b1 go absolutely first — the
            # dec matmuls stream behind the arriving w1d chunks.  Then the
            # first main psum tile's inputs (w1e8 c0 + ht8 j0), then the
            # rest of b0's j0 deps, then everything else.
            w1e8_t = [const.tile([128, U], fp8, tag=f"w1e8{c}", name=f"w1e8_t{c}") for c in range(NF8)]
            w1eb_t = [const.tile([128, U], bf, tag=f"w1eb{c}", name=f"w1eb_t{c}") for c in range(NC_ - NF8)]
            ht0_8t = [hpool.tile([128, JW], fp8, tag="ht8", name=f"ht0_8t{j}") for j in range(NJ)]
            ht0_bt = [hpool.tile([128, JW], bf, tag="htb", name=f"ht0_bt{j}") for j in range(NJ)]
            w1d_t = [const.tile([128, U], bf, tag=f"w1d{c}", name=f"w1d_t{c}") for c in range(NC_)]
            nc.sync.dma_start(w1d_t[0][:], w1d_p[:, 0:U])
            hdect_sb = const.tile([128, NC_ * BL], bf)
            nc.sync.dma_start(hdect_sb[:], hdect_p[:])
            b1_sb = const.tile([128, NC_], f32)
            nc.sync.dma_start(b1_sb[:], b1_p[:])
            for c in (1, 2, 3):
                nc.sync.dma_start(w1d_t[c][:], w1d_p[:, c * U : (c + 1) * U])
            w2_sb = const.tile([128, NC_], bf)
            nc.sync.dma_start(w2_sb[:], w2_p[:])
            b2_sb = const.tile([1, 1], f32)
            nc.sync.dma_start(b2_sb[:], b2_p[:])
            ones_sb = const.tile([1, 128], bf)
            nc.sync.dma_start(ones_sb[:], ones_p[:])
            onesf_sb = const.tile([1, 128], f32)
            nc.sync.dma_start(onesf_sb[:], onesf_p[:])
            nc.sync.dma_start(w1e8_t[0][:], w1e8_p[:, 0:U])
            for piece in range(4):
                nc.sync.dma_start(
                    ht0_8t[0][:, piece * 1024 : (piece + 1) * 1024],
                    ht8_p[0][:, piece * 1024 : (piece + 1) * 1024],
                )
            for c in range(4, NC_):
                nc.sync.dma_start(w1d_t[c][:], w1d_p[:, c * U : (c + 1) * U])
            for piece in range(4):
                nc.sync.dma_start(
                    ht0_bt[0][:, piece * 1024 : (piece + 1) * 1024],
                    htb_p[0][:, piece * 1024 : (piece + 1) * 1024],
                )
            # secondary startup bytes go through SWDGE (gpsimd) queues —
            # separate from the HWDGE queues carrying the critical set, so
            # the two pools add bandwidth during the ramp.
            for c in range(NC_ - NF8):
                nc.gpsimd.dma_start(w1eb_t[c][:], w1eb_p[:, c * U : (c + 1) * U])
            for c in range(1, NF8):
                nc.gpsimd.dma_start(w1e8_t[c][:], w1e8_p[:, c * U : (c + 1) * U])
            for j in range(1, NJ):
                nc.sync.dma_start(ht0_8t[j][:], ht8_p[0][:, j * JW : (j + 1) * JW])
                nc.sync.dma_start(ht0_bt[j][:], htb_p[0][:, j * JW : (j + 1) * JW])

            # ---- dec_proj^T + b1 -> bias_t[c][p, b] for u = c*128+p ----
            # Per-c bias tiles so tanh(c) only waits on its own chunk; dec
            # chunks c>=3 are emitted interleaved with the first main-matmul
            # sweep so the PE isn't stalled behind late w1d DMA chunks.
            bias_t = [
                const.tile([128, BL], f32, tag=f"bias{c}", name=f"bias_t{c}")
                for c in range(NC_)
            ]

            def emit_dec(c):
                ps_dec = psA.tile([128, BL], f32, tag="e1", name=f"psdec{c}")
                for k in range(NC_):
                    nc.tensor.matmul(
                        ps_dec[:],
                        w1d_t[c][:, k * 128 : (k + 1) * 128],
                        hdect_sb[:, k * BL : (k + 1) * BL],
                        start=(k == 0),
                        stop=(k == NC_ - 1),
                    )
                nc.vector.tensor_scalar_add(
                    bias_t[c][:], ps_dec[:], b1_sb[:, c : c + 1]
                )

            for c in range(3):
                emit_dec(c)

            # ---- per-batch pipeline ----
            ctx_all = const.tile([128, BL * NC_], f32)
            for b in range(BL):
                if b == 0:
                    ht8_t, htb_t = ht0_8t, ht0_bt
                else:
                    # j-major host layout -> each j-slice is contiguous;
                    # compute on s-chunk j starts as soon as its slice lands.
                    ht8_t = [hpool.tile([128, JW], fp8, tag="ht8", name=f"ht8_t{b}_{j}") for j in range(NJ)]
                    htb_t = [hpool.tile([128, JW], bf, tag="htb", name=f"htb_t{b}_{j}") for j in range(NJ)]
                    for j in range(NJ):
                        nc.sync.dma_start(
                            ht8_t[j][:], ht8_p[b][:, j * JW : (j + 1) * JW]
                        )
                    for j in range(NJ):
                        nc.sync.dma_start(
                            htb_t[j][:], htb_p[b][:, j * JW : (j + 1) * JW]
                        )

                exp_row = rows.tile([1, S], f32, tag="exp")
                zparts = rows.tile([1, NJ], f32, tag="zp")
                # per-(d-chunk, j-chunk) partial context sums with the
                # UNNORMALIZED exp weights — ctx work for chunk j overlaps
                # the matmuls of chunks j+1..; normalized once at the end.
                ctx_parts = rows.tile([128, NC_ * NJ], f32, tag="cparts")

                for j in range(NJ):
                    ps_e2 = psB.tile([1, S512], f32, tag="e2")
                    ht_j3 = ht8_t[j].rearrange("p (k f) -> p k f", k=NC_, f=S512)
                    for c in range(NC_):
                        ps_e1 = psA.tile([128, S512], f32, tag="e1")
                        if c < NF8:
                            w1e_c3 = w1e8_t[c].rearrange(
                                "p (k m) -> p k m", k=NC_, m=128
                            )
                            for k2 in range(NC_ // 2):
                                nc.tensor.matmul(
                                    ps_e1[:],
                                    w1e_c3[:, 2 * k2 : 2 * k2 + 2, :],
                                    ht_j3[:, 2 * k2 : 2 * k2 + 2, :],
                                    start=(k2 == 0),
                                    stop=(k2 == NC_ // 2 - 1),
                                    perf_mode=mybir.MatmulPerfMode.DoubleRow,
                                )
                        else:
                            for k in range(NC_):
                                nc.tensor.matmul(
                                    ps_e1[:],
                                    w1eb_t[c - NF8][:, k * 128 : (k + 1) * 128],
                                    htb_t[j][:, k * S512 : (k + 1) * S512],
                                    start=(k == 0),
                                    stop=(k == NC_ - 1),
                                )
                        if b == 0 and j == 0 and c < 5:
                            emit_dec(c + 3)
                        e1_sb = e1pool.tile([128, S512], bf, tag="e1sb")
                        nc.scalar.activation(
                            e1_sb[:],
                            ps_e1[:],
                            AF.Tanh,
                            bias=bias_t[c][:, b : b + 1],
                            scale=(1.0 / W1E_SCALE) if c < NF8 else 1.0,
                        )
                        nc.tensor.matmul(
                            ps_e2[:],
                            w2_sb[:, c : c + 1],
                            e1_sb[:],
                            start=(c == 0),
                            stop=(c == NC_ - 1),
                        )
                    # relu(e2 + b2), then exp with per-chunk partial sum
                    r_row = rows.tile([1, S512], f32, tag="rrow")
                    nc.scalar.activation(
                        r_row[:], ps_e2[:], AF.Relu, bias=b2_sb[0:1, 0:1], scale=1.0
                    )
                    nc.scalar.activation(
                        exp_row[:, j * S512 : (j + 1) * S512],
                        r_row[:],
                        AF.Exp,
                        accum_out=zparts[:, j : j + 1],
                    )
                    # broadcast unnormalized exp chunk to 128 partitions
                    wn_bj = rows.tile([1, S512], bf, tag="wnbj")
                    nc.vector.tensor_copy(
                        wn_bj[:], exp_row[:, j * S512 : (j + 1) * S512]
                    )
                    ps_wb = psC.tile([128, S512], f32, tag="wb")
                    nc.tensor.matmul(
                        ps_wb[:], ones_sb[:], wn_bj[:], start=True, stop=True
                    )
                    wb_j = wbpool.tile([128, S512], bf, tag="wb")
                    nc.scalar.copy(wb_j[:], ps_wb[:])
                    # fused multiply + accumulate per d-chunk
                    for t in range(NC_):
                        scratch = scrpool.tile([128, S512], bf, tag="scr")
                        nc.vector.scalar_tensor_tensor(
                            out=scratch[:],
                            in0=htb_t[j][:, t * S512 : (t + 1) * S512],
                            scalar=1.0,
                            in1=wb_j[:],
                            op0=ALU.mult,
                            op1=ALU.mult,
                            accum_out=ctx_parts[:, t * NJ + j : t * NJ + j + 1],
                        )

                # softmax normalization scalars
                z = rows.tile([1, 1], f32, tag="z")
                nc.vector.reduce_sum(out=z[:], in_=zparts[:], axis=AX.X)
                zinv = rows.tile([1, 1], f32, tag="zi")
                nc.vector.reciprocal(zinv[:], z[:])
                # attention weights out (normalized rows)
                wn_f = rows.tile([1, S], f32, tag="wnf")
                nc.vector.tensor_scalar_mul(wn_f[:], exp_row[:], zinv[0:1, 0:1])
                nc.sync.dma_start(attn_p[b : b + 1, :], wn_f[:])
                # broadcast 1/Z down partitions (f32 ones-matmul, exact)
                ps_zb = psC.tile([128, 1], f32, tag="wb")
                nc.tensor.matmul(
                    ps_zb[:], onesf_sb[:], zinv[:], start=True, stop=True
                )
                zcol = rows.tile([128, 1], f32, tag="zcol")
                nc.scalar.copy(zcol[:], ps_zb[:])
                # reduce partials over j, scale by 1/Z, store
                ctx_red = rows.tile([128, NC_], f32, tag="ctx")
                nc.vector.reduce_sum(
                    out=ctx_red[:],
                    in_=ctx_parts.rearrange("p (t j) -> p t j", t=NC_, j=NJ),
                    axis=AX.X,
                )
                nc.vector.tensor_scalar_mul(
                    ctx_all[:, b * NC_ : (b + 1) * NC_], ctx_red[:], zcol[:]
                )
                if b == BL - 1:
                    nc.sync.dma_start(ctx_p[:], ctx_all[:])

    _dedup_ldweights(nc, mybir)
    _split_multi_waits(nc, mybir)
    return nc


def _prep_inputs(h_encoder, h_decoder_prev, W1, b1, W2, b2):
    import ml_dtypes

    bf = ml_dtypes.bfloat16

    W1e = np.ascontiguousarray(W1[:D])  # [D, U]
    W1d = np.ascontiguousarray(W1[D:])  # [D, U]
    # Permute the (contracted-away) u-axis so the u's with smallest |W2|
    # land in the fp8 chunks — they carry the least error into e2.
    # Outputs are invariant to this permutation.
    perm = np.argsort(np.abs(np.asarray(W2)[:, 0]), kind="stable")
    W1e = W1e[:, perm]
    W1d = W1d[:, perm]
    W2 = np.asarray(W2)[perm]
    b1 = np.asarray(b1)[perm]
    fp8 = ml_dtypes.float8_e4m3
    # c-major layout: [p, c*U + k*128 + m] = W[k*128 + p, c*128 + m]
    # fp8 chunks scaled x32 so e4m3 sees ~N(0, 0.64); descaled in tanh
    w1e_cm = W1e.reshape(NC_, 128, NC_, 128).transpose(1, 2, 0, 3).reshape(
        128, NC_ * U
    )
    w1e8_r = (w1e_cm[:, : NF8 * U] * W1E_SCALE).astype(fp8)
    w1eb_r = w1e_cm[:, NF8 * U :].astype(bf)
    w1d_r = (
        W1d.reshape(NC_, 128, NC_, 128)
        .transpose(1, 2, 0, 3)
        .reshape(128, NC_ * U)
        .astype(bf)
    )
    w2_r = np.ascontiguousarray(W2[:, 0]).reshape(NC_, 128).transpose(1, 0).astype(bf)
    b1_r = np.ascontiguousarray(b1).reshape(NC_, 128).transpose(1, 0).astype(np.float32)
    b2_r = np.asarray(b2, np.float32).reshape(1, 1)
    ones = np.ones((1, 128), bf)
    onesf = np.ones((1, 128), np.float32)

    in_maps = []
    for i in range(N_CORES):
        hs = h_encoder[i * BL : (i + 1) * BL]  # [BL, S, D]
        # -> [b, p, j*4096 + k*512 + f] with d = k*128 + p, s = j*512 + f
        ht_lay = (
            hs.transpose(0, 2, 1)  # [BL, D, S]
            .reshape(BL, NC_, 128, NJ, S512)  # [b, k, p, j, f]
            .transpose(0, 2, 3, 1, 4)  # [b, p, j, k, f]
        )
        ht8 = ht_lay.astype(fp8).reshape(BL, 128, NC_ * S)
        htb = ht_lay.astype(bf).reshape(BL, 128, NC_ * S)
        hd = h_decoder_prev[i * BL : (i + 1) * BL]  # [BL, D]
        hdect = (
            hd.transpose(1, 0)  # [D, BL]
            .reshape(NC_, 128, BL)
            .transpose(1, 0, 2)  # [128, NC_, BL]
            .astype(bf)
            .reshape(128, NC_ * BL)
        )
        in_maps.append(
            {
                "ht8": ht8,
                "htb": htb,
                "hdect": hdect,
                "w1e8": w1e8_r,
                "w1eb": w1eb_r,
                "w1d": w1d_r,
                "w2": w2_r,
                "b1": b1_r,
                "b2": b2_r,
                "ones": ones,
                "onesf": onesf,
            }
        )
    return in_maps


_CACHE = {}


def kernel(h_encoder, h_decoder_prev, W1, b1, W2, b2):
    from concourse.bass_utils import run_bass_kernel_spmd

    if "nc" not in _CACHE:
        _CACHE["nc"] = _build_nc()
    nc = _CACHE["nc"]

    in_maps = _prep_inputs(h_encoder, h_decoder_prev, W1, b1, W2, b2)
    res = run_bass_kernel_spmd(nc, in_maps, list(range(N_CORES)))

    ctx = np.concatenate(
        [
            res.results[i]["ctx"].reshape(128, BL, NC_).transpose(1, 2, 0).reshape(BL, D)
            for i in range(N_CORES)
        ],
        axis=0,
    ).astype(np.float32)
    attn = np.concatenate(
        [res.results[i]["attn"] for i in range(N_CORES)], axis=0
    ).astype(np.float32).reshape(B, S, 1)
    return (ctx, attn)


# revision 55
# speedup vs baseline: 1.0141x; 1.0141x over previous
"""Bahdanau-attention kernel for Trainium2, data-parallel over batch on 8 cores.

Shapes (full): h_encoder [32, 2048, 1024], h_decoder_prev [32, 1024],
W1 [2048, 1024], b1 [1024], W2 [1024, 1], b2 [1].
Returns (context_vector [32, 1024] f32, attention_weights [32, 2048, 1] f32).

Per-core (4 batch elements each), all matmuls in bf16 with f32 PSUM accum:
  E1^T[u,s] = tanh(W1_enc^T . h^T + dec_proj[b] + b1)   (TensorE + ScalarE)
  e2[s]     = relu(W2^T . E1^T + b2)                    (TensorE + ScalarE)
  w[s]      = softmax_s(e2)                             (ScalarE exp + VectorE)
  ctx[d]    = sum_s w[s] h[b,s,d]                       (VectorE mult-reduce on h^T)

Host prep (cheap, no FLOP-relevant work): shard by batch, transpose
h_encoder to [d, s] per batch, split W1, cast to bf16.
"""

import numpy as np

B, S, D, U = 32, 2048, 1024, 1024
W1E_SCALE = 32.0
NF8 = 6  # u-chunks computed in fp8 DoubleRow; rest bf16
N_CORES = 8
BL = B // N_CORES  # 4 batch elements per core
S512 = 512
NJ = S // S512  # 4 s-chunks
NC_ = 8  # number of 128-wide chunks in D / U


def _patch_tile_drain(tile, mybir):
    """This walrus build rejects >1 sync wait on one CTRL instruction; split
    the Tile tail-drain's waits one-per-nop."""
    from concourse.vector_clock import ScopedClock

    if getattr(tile.TileContext, "_drain_patched", False):
        return

    def _patched(self, tick_clock, wait_clock):
        nc = self.nc
        drain_inst = nc.sync.drain()
        wait_clock.add_sem_waits(
            drain_inst.ins, ScopedClock({None: tick_clock.global_clock})
        )
        si = drain_inst.ins.sync_info
        if si is not None and si.on_wait and len(si.on_wait) > 1:
            extras = list(si.on_wait[1:])
            si.on_wait = [si.on_wait[0]]
            for w in extras:
                nop = nc.sync.nop(nofuse=True, hint="drain_wait_split")
                nsi = nop.ins.sync_info
                if nsi is None:
                    nop.ins.sync_info = mybir.SyncInfo(on_wait=[w], on_update=[])
                else:
                    nsi.on_wait = [w]
        nc.all_engine_barrier()
        popped = nc._tile_sem_poison_stack.pop()
        assert popped is self._sem_poison
        # clear_and_free_semaphores emits one RANGE_CLEAR ISA instruction for
        # the whole range, which this walrus rejects ("ISA wrong length") for
        # wide ranges — chunk it.
        sems = list(self.sems.allocated().values())
        sem_nums = sorted(
            s.num if hasattr(s, "num") else int(s) for s in sems
        )
        CH = 8
        for i in range(0, len(sem_nums), CH):
            chunk = sem_nums[i : i + CH]
            from concourse.bass import compact_to_ranges

            for r in compact_to_ranges(chunk):
                nc.gpsimd.dma_reset(r)
                nc.gpsimd.sem_clear(r)

    tile.TileContext._drain_and_barrier = _patched
    tile.TileContext._drain_patched = True


_COMPUTE_ENGINES = ("PE", "Activation", "DVE", "Pool", "SP")


def _dedup_ldweights(nc, mybir):
    """Drop an LDWEIGHTS that reloads the exact weights of the immediately
    preceding LDWEIGHTS on the PE stream.  The following matmul then uses the
    already-loaded stationary (HW preserves program-order weight semantics).
    Only drops sync-free instances."""
    dropped = 0
    for fn in nc.m.functions:
        for bb in fn.blocks:
            last_sig = None
            keep = []
            for inst in bb.instructions:
                nm = type(inst).__name__
                if nm == "InstLdweights":
                    sig = (
                        str(inst.ins[0]),
                        str(getattr(inst, "perf_mode", None)),
                        str(getattr(inst, "is_transpose", None)),
                    )
                    si = inst.sync_info
                    clean = si is None or (not si.on_wait and not si.on_update)
                    if sig == last_sig and clean:
                        dropped += 1
                        continue
                    last_sig = sig
                keep.append(inst)
            if len(keep) != len(bb.instructions):
                bb.instructions[:] = keep
    return dropped


def _split_multi_waits(nc, mybir, max_waits=1):
    """This walrus build rejects instructions carrying more than one sync
    wait; hoist extras onto nops inserted just before, on the same engine."""
    idx = 0
    eng_ok = {getattr(mybir.EngineType, n) for n in _COMPUTE_ENGINES}
    for fn in nc.m.functions:
        for bb in fn.blocks:
            new_insts = []
            changed = False
            for inst in bb.instructions:
                si = inst.sync_info
                if (
                    si is not None
                    and si.on_wait
                    and len(si.on_wait) > max_waits
                    and inst.engine in eng_ok
                ):
                    waits = list(si.on_wait)
                    extras, keep = waits[:-max_waits], waits[-max_waits:]
                    for w in extras:
                        idx += 1
                        new_insts.append(
                            mybir.InstNoOp(
                                name=nc.get_next_instruction_name(),
                                engine=inst.engine,
                                ins=[],
                                outs=[],
                                sync_info=mybir.SyncInfo(on_wait=[w], on_update=[]),
                                bass_nofuse=True,
                                text_hint="waitsplit",
                            )
                        )
                    si.on_wait = keep
                    changed = True
                new_insts.append(inst)
            if changed:
                try:
                    bb.instructions[:] = new_insts
                except TypeError:
                    bb.instructions = new_insts


def _build_nc():
    import concourse.bass as bass
    import concourse.mybir as mybir
    import concourse.tile as tile

    _patch_tile_drain(tile, mybir)

    bf = mybir.dt.bfloat16
    f32 = mybir.dt.float32
    fp8 = mybir.dt.float8e4
    AF = mybir.ActivationFunctionType
    ALU = mybir.AluOpType
    AX = mybir.AxisListType

    nc = bass.Bass()

    ht8_p = nc.declare_dram_parameter("ht8", [BL, 128, NC_ * S], fp8, isOutput=False)
    htb_p = nc.declare_dram_parameter("htb", [BL, 128, NC_ * S], bf, isOutput=False)
    hdect_p = nc.declare_dram_parameter("hdect", [128, NC_ * BL], bf, isOutput=False)
    w1e8_p = nc.declare_dram_parameter("w1e8", [128, NF8 * U], fp8, isOutput=False)
    w1eb_p = nc.declare_dram_parameter("w1eb", [128, (NC_ - NF8) * U], bf, isOutput=False)
    w1d_p = nc.declare_dram_parameter("w1d", [128, NC_ * U], bf, isOutput=False)
    w2_p = nc.declare_dram_parameter("w2", [128, NC_], bf, isOutput=False)
    b1_p = nc.declare_dram_parameter("b1", [128, NC_], f32, isOutput=False)
    b2_p = nc.declare_dram_parameter("b2", [1, 1], f32, isOutput=False)
    ones_p = nc.declare_dram_parameter("ones", [1, 128], bf, isOutput=False)
    onesf_p = nc.declare_dram_parameter("onesf", [1, 128], f32, isOutput=False)
    ctx_p = nc.declare_dram_parameter("ctx", [128, BL * NC_], f32, isOutput=True)
    attn_p = nc.declare_dram_parameter("attn", [BL, S], f32, isOutput=True)

    with tile.TileContext(nc) as tc:
        with (
            tc.tile_pool(name="const", bufs=1) as const,
            tc.tile_pool(name="hpool", bufs=8) as hpool,
            tc.tile_pool(name="e1pool", bufs=4) as e1pool,
            tc.tile_pool(name="wbpool", bufs=2) as wbpool,
            tc.tile_pool(name="scrpool", bufs=2) as scrpool,
            tc.tile_pool(name="rows", bufs=2) as rows,
            tc.tile_pool(name="psA", bufs=4, space="PSUM") as psA,
            tc.tile_pool(name="psB", bufs=2, space="PSUM") as psB,
            tc.tile_pool(name="psC", bufs=2, space="PSUM") as psC,
        ):
            # ---- constants into SBUF ----
            # Weights are laid out c-major on host ([p, c*U + k*128 + m]) so
            # one 256KB chunk carries a full contraction for one u-chunk.
            # DMA issue order = first-compute order: the first psum tile only
            # needs w1e chunk c0 + ht(b0) slice j0 (~1.25MB), so those go
            # first; then the dec_proj deps, then the rest.
            JW = NC_ * S512  # 4096: width of one j-major slice
            # Deps are effectively tile-granular, so give every
            # independently-consumed chunk its own tile.  The first psum tile
            # needs w1e chunk c0 + ht(b0) slice j0 (~1.25MB): issue those
            # first, split across several dma_starts so all HWDGE queues
            # work on the critical bytes before anything else is queued.
            # dec_proj heads the PE stream and the first tanh needs its
            # result (bias), so w1d + hdect + b1 go absolutely first — the
            # dec matmuls stream behind the arriving w1d chunks.  Then the
            # first main psum tile's inputs (w1e8 c0 + ht8 j0), then the
            # rest of b0's j0 deps, then everything else.
            w1e8_t = [const.tile([128, U], fp8, tag=f"w1e8{c}", name=f"w1e8_t{c}") for c in range(NF8)]
            w1eb_t = [const.tile([128, U], bf, tag=f"w1eb{c}", name=f"w1eb_t{c}") for c in range(NC_ - NF8)]
            ht0_8t = [hpool.tile([128, JW], fp8, tag="ht8", name=f"ht0_8t{j}") for j in range(NJ)]
            ht0_bt = [hpool.tile([128, JW], bf, tag="htb", name=f"ht0_bt{j}") for j in range(NJ)]
            w1d_t = [const.tile([128, U], bf, tag=f"w1d{c}", name=f"w1d_t{c}") for c in range(NC_)]
            nc.sync.dma_start(w1d_t[0][:], w1d_p[:, 0:U])
            hdect_sb = const.tile([128, NC_ * BL], bf)
            nc.sync.dma_start(hdect_sb[:], hdect_p[:])
            b1_sb = const.tile([128, NC_], f32)
            nc.sync.dma_start(b1_sb[:], b1_p[:])
            for c in (1, 2, 3):
                nc.sync.dma_start(w1d_t[c][:], w1d_p[:, c * U : (c + 1) * U])
            w2_sb = const.tile([128, NC_], bf)
            nc.sync.dma_start(w2_sb[:], w2_p[:])
            b2_sb = const.tile([1, 1], f32)
            nc.sync.dma_start(b2_sb[:], b2_p[:])
            ones_sb = const.tile([1, 128], bf)
            nc.sync.dma_start(ones_sb[:], ones_p[:])
            onesf_sb = const.tile([1, 128], f32)
            nc.sync.dma_start(onesf_sb[:], onesf_p[:])
            nc.sync.dma_start(w1e8_t[0][:], w1e8_p[:, 0:U])
            for piece in range(4):
                nc.sync.dma_start(
                    ht0_8t[0][:, piece * 1024 : (piece + 1) * 1024],
                    ht8_p[0][:, piece * 1024 : (piece + 1) * 1024],
                )
            for c in range(4, NC_):
                nc.sync.dma_start(w1d_t[c][:], w1d_p[:, c * U : (c + 1) * U])
            for piece in range(4):
                nc.sync.dma_start(
                    ht0_bt[0][:, piece * 1024 : (piece + 1) * 1024],
                    htb_p[0][:, piece * 1024 : (piece + 1) * 1024],
                )
            # secondary startup bytes go through SWDGE (gpsimd) queues —
            # separate from the HWDGE queues carrying the critical set, so
            # the two pools add bandwidth during the ramp.
            for c in range(NC_ - NF8):
                nc.gpsimd.dma_start(w1eb_t[c][:], w1eb_p[:, c * U : (c + 1) * U])
            for c in range(1, NF8):
                nc.gpsimd.dma_start(w1e8_t[c][:], w1e8_p[:, c * U : (c + 1) * U])
            for j in range(1, NJ):
                nc.sync.dma_start(ht0_8t[j][:], ht8_p[0][:, j * JW : (j + 1) * JW])
                nc.sync.dma_start(ht0_bt[j][:], htb_p[0][:, j * JW : (j + 1) * JW])

            # ---- dec_proj^T + b1 -> bias_t[c][p, b] for u = c*128+p ----
            # Per-c bias tiles so tanh(c) only waits on its own chunk; dec
            # chunks c>=3 are emitted interleaved with the first main-matmul
            # sweep so the PE isn't stalled behind late w1d DMA chunks.
            bias_t = [
                const.tile([128, BL], f32, tag=f"bias{c}", name=f"bias_t{c}")
                for c in range(NC_)
            ]

            def emit_dec(c):
                ps_dec = psA.tile([128, BL], f32, tag="e1", name=f"psdec{c}")
                for k in range(NC_):
                    nc.tensor.matmul(
                        ps_dec[:],
                        w1d_t[c][:, k * 128 : (k + 1) * 128],
                        hdect_sb[:, k * BL : (k + 1) * BL],
                        start=(k == 0),
                        stop=(k == NC_ - 1),
                    )
                nc.vector.tensor_scalar_add(
                    bias_t[c][:], ps_dec[:], b1_sb[:, c : c + 1]
                )

            for c in range(3):
                emit_dec(c)

            # ---- per-batch pipeline ----
            ctx_all = const.tile([128, BL * NC_], f32)
            for b in range(BL):
                if b == 0:
                    ht8_t, htb_t = ht0_8t, ht0_bt
                else:
                    # j-major host layout -> each j-slice is contiguous;
                    # compute on s-chunk j starts as soon as its slice lands.
                    ht8_t = [hpool.tile([128, JW], fp8, tag="ht8", name=f"ht8_t{b}_{j}") for j in range(NJ)]
                    htb_t = [hpool.tile([128, JW], bf, tag="htb", name=f"htb_t{b}_{j}") for j in range(NJ)]
                    for j in range(NJ):
                        nc.sync.dma_start(
                            ht8_t[j][:], ht8_p[b][:, j * JW : (j + 1) * JW]
                        )
                    for j in range(NJ):
                        nc.sync.dma_start(
                            htb_t[j][:], htb_p[b][:, j * JW : (j + 1) * JW]
                        )

                exp_row = rows.tile([1, S], f32, tag="exp")
                zparts = rows.tile([1, NJ], f32, tag="zp")
                # per-(d-chunk, j-chunk) partial context sums with the
                # UNNORMALIZED exp weights — ctx work for chunk j overlaps
                # the matmuls of chunks j+1..; normalized once at the end.
                ctx_parts = rows.tile([128, NC_ * NJ], f32, tag="cparts")

                for j in range(NJ):
                    ps_e2 = psB.tile([1, S512], f32, tag="e2")
                    ht_j3 = ht8_t[j].rearrange("p (k f) -> p k f", k=NC_, f=S512)
                    for c in range(NC_):
                        ps_e1 = psA.tile([128, S512], f32, tag="e1")
                        if c < NF8:
                            w1e_c3 = w1e8_t[c].rearrange(
                                "p (k m) -> p k m", k=NC_, m=128
                            )
                            for k2 in range(NC_ // 2):
                                nc.tensor.matmul(
                                    ps_e1[:],
                                    w1e_c3[:, 2 * k2 : 2 * k2 + 2, :],
                                    ht_j3[:, 2 * k2 : 2 * k2 + 2, :],
                                    start=(k2 == 0),
                                    stop=(k2 == NC_ // 2 - 1),
                                    perf_mode=mybir.MatmulPerfMode.DoubleRow,
                                )
                        else:
                            for k in range(NC_):
                                nc.tensor.matmul(
                                    ps_e1[:],
                                    w1eb_t[c - NF8][:, k * 128 : (k + 1) * 128],
                                    htb_t[j][:, k * S512 : (k + 1) * S512],
                                    start=(k == 0),
                                    stop=(k == NC_ - 1),
                                )
                        if b == 0 and j == 0 and c < 5:
                            emit_dec(c + 3)
                        e1_sb = e1pool.tile([128, S512], bf, tag="e1sb")
                        nc.scalar.activation(
                            e1_sb[:],
                            ps_e1[:],
                            AF.Tanh,
                            bias=bias_t[c][:, b : b + 1],
                            scale=(1.0 / W1E_SCALE) if c < NF8 else 1.0,
                        )
                        nc.tensor.matmul(
                            ps_e2[:],
                            w2_sb[:, c : c + 1],
                            e1_sb[:],
                            start=(c == 0),
                            stop=(c == NC_ - 1),
                        )
                    # relu(e2 + b2), then exp with per-chunk partial sum
                    r_row = rows.tile([1, S512], f32, tag="rrow")
                    nc.scalar.activation(
                        r_row[:], ps_e2[:], AF.Relu, bias=b2_sb[0:1, 0:1], scale=1.0
                    )
                    nc.scalar.activation(
                        exp_row[:, j * S512 : (j + 1) * S512],
                        r_row[:],
                        AF.Exp,
                        accum_out=zparts[:, j : j + 1],
                    )
                    # broadcast unnormalized exp chunk to 128 partitions
                    wn_bj = rows.tile([1, S512], bf, tag="wnbj")
                    nc.vector.tensor_copy(
                        wn_bj[:], exp_row[:, j * S512 : (j + 1) * S512]
                    )
                    ps_wb = psC.tile([128, S512], f32, tag="wb")
                    nc.tensor.matmul(
                        ps_wb[:], ones_sb[:], wn_bj[:], start=True, stop=True
                    )
                    wb_j = wbpool.tile([128, S512], bf, tag="wb")
                    nc.scalar.copy(wb_j[:], ps_wb[:])
                    # fused multiply + accumulate per d-chunk
                    for t in range(NC_):
                        scratch = scrpool.tile([128, S512], bf, tag="scr")
                        nc.vector.scalar_tensor_tensor(
                            out=scratch[:],
                            in0=htb_t[j][:, t * S512 : (t + 1) * S512],
                            scalar=1.0,
                            in1=wb_j[:],
                            op0=ALU.mult,
                            op1=ALU.mult,
                            accum_out=ctx_parts[:, t * NJ + j : t * NJ + j + 1],
                        )

                # softmax normalization scalars
                z = rows.tile([1, 1], f32, tag="z")
                nc.vector.reduce_sum(out=z[:], in_=zparts[:], axis=AX.X)
                zinv = rows.tile([1, 1], f32, tag="zi")
                nc.vector.reciprocal(zinv[:], z[:])
                # attention weights out (normalized rows)
                wn_f = rows.tile([1, S], f32, tag="wnf")
                nc.vector.tensor_scalar_mul(wn_f[:], exp_row[:], zinv[0:1, 0:1])
                nc.sync.dma_start(attn_p[b : b + 1, :], wn_f[:])
                # broadcast 1/Z down partitions (f32 ones-matmul, exact)
                ps_zb = psC.tile([128, 1], f32, tag="wb")
                nc.tensor.matmul(
                    ps_zb[:], onesf_sb[:], zinv[:], start=True, stop=True
                )
                zcol = rows.tile([128, 1], f32, tag="zcol")
                nc.scalar.copy(zcol[:], ps_zb[:])
                # reduce partials over j, scale by 1/Z, store
                ctx_red = rows.tile([128, NC_], f32, tag="ctx")
                nc.vector.reduce_sum(
                    out=ctx_red[:],
                    in_=ctx_parts.rearrange("p (t j) -> p t j", t=NC_, j=NJ),
                    axis=AX.X,
                )
                nc.vector.tensor_scalar_mul(
                    ctx_all[:, b * NC_ : (b + 1) * NC_], ctx_red[:], zcol[:]
                )
                if b == BL - 1:
                    nc.sync.dma_start(ctx_p[:], ctx_all[:])

    _dedup_ldweights(nc, mybir)
    _split_multi_waits(nc, mybir)
    return nc


def _prep_inputs(h_encoder, h_decoder_prev, W1, b1, W2, b2):
    import ml_dtypes

    bf = ml_dtypes.bfloat16

    W1e = np.ascontiguousarray(W1[:D])  # [D, U]
    W1d = np.ascontiguousarray(W1[D:])  # [D, U]
    # Permute the (contracted-away) u-axis so the u's with smallest |W2|
    # land in the fp8 chunks — they carry the least error into e2.
    # Outputs are invariant to this permutation.
    perm = np.argsort(np.abs(np.asarray(W2)[:, 0]), kind="stable")
    W1e = W1e[:, perm]
    W1d = W1d[:, perm]
    W2 = np.asarray(W2)[perm]
    b1 = np.asarray(b1)[perm]
    fp8 = ml_dtypes.float8_e4m3
    # c-major layout: [p, c*U + k*128 + m] = W[k*128 + p, c*128 + m]
    # fp8 chunks scaled x32 so e4m3 sees ~N(0, 0.64); descaled in tanh
    w1e_cm = W1e.reshape(NC_, 128, NC_, 128).transpose(1, 2, 0, 3).reshape(
        128, NC_ * U
    )
    w1e8_r = (w1e_cm[:, : NF8 * U] * W1E_SCALE).astype(fp8)
    w1eb_r = w1e_cm[:, NF8 * U :].astype(bf)
    w1d_r = (
        W1d.reshape(NC_, 128, NC_, 128)
        .transpose(1, 2, 0, 3)
        .reshape(128, NC_ * U)
        .astype(bf)
    )
    w2_r = np.ascontiguousarray(W2[:, 0]).reshape(NC_, 128).transpose(1, 0).astype(bf)
    b1_r = np.ascontiguousarray(b1).reshape(NC_, 128).transpose(1, 0).astype(np.float32)
    b2_r = np.asarray(b2, np.float32).reshape(1, 1)
    ones = np.ones((1, 128), bf)
    onesf = np.ones((1, 128), np.float32)

    in_maps = []
    for i in range(N_CORES):
        hs = h_encoder[i * BL : (i + 1) * BL]  # [BL, S, D]
        # -> [b, p, j*4096 + k*512 + f] with d = k*128 + p, s = j*512 + f
        ht_lay = (
            hs.transpose(0, 2, 1)  # [BL, D, S]
            .reshape(BL, NC_, 128, NJ, S512)  # [b, k, p, j, f]
            .transpose(0, 2, 3, 1, 4)  # [b, p, j, k, f]
        )
        ht8 = ht_lay.astype(fp8).reshape(BL, 128, NC_ * S)
        htb = ht_lay.astype(bf).reshape(BL, 128, NC_ * S)
        hd = h_decoder_prev[i * BL : (i + 1) * BL]  # [BL, D]
        hdect = (
            hd.transpose(1, 0)  # [D, BL]
            .reshape(NC_, 128, BL)
            .transpose(1, 0, 2)  # [128, NC_, BL]
            .astype(bf)
            .reshape(128, NC_ * BL)
        )
        in_maps.append(
            {
                "ht8": ht8,
                "htb": htb,
                "hdect": hdect,
                "w1e8": w1e8_r,
                "w1eb": w1eb_r,
                "w1d": w1d_r,
                "w2": w2_r,
                "b1": b1_r,
                "b2": b2_r,
                "ones": ones,
                "onesf": onesf,
            }
        )
    return in_maps


_CACHE = {}


def kernel(h_encoder, h_decoder_prev, W1, b1, W2, b2):
    from concourse.bass_utils import run_bass_kernel_spmd

    if "nc" not in _CACHE:
        _CACHE["nc"] = _build_nc()
    nc = _CACHE["nc"]

    in_maps = _prep_inputs(h_encoder, h_decoder_prev, W1, b1, W2, b2)
    res = run_bass_kernel_spmd(nc, in_maps, list(range(N_CORES)))

    ctx = np.concatenate(
        [
            res.results[i]["ctx"].reshape(128, BL, NC_).transpose(1, 2, 0).reshape(BL, D)
            for i in range(N_CORES)
        ],
        axis=0,
    ).astype(np.float32)
    attn = np.concatenate(
        [res.results[i]["attn"] for i in range(N_CORES)], axis=0
    ).astype(np.float32).reshape(B, S, 1)
    return (ctx, attn)
